# revision 19
# baseline (speedup 1.0000x reference)
"""DKVMN Trainium2 kernel (self-contained).

Strategy (per core, data-parallel over batch B=256 -> 32/core):
  The DKVMN step  mem = mem*(1 - w x e) + w x a  is a first-order affine
  recurrence per (b, m, d) scalar:  x_t = A_t x_{t-1} + B_t  with
  A = 1 - w*e, B = w*a.  The gates w (softmax), e (sigmoid), a (tanh) do not
  depend on mem, so they are precomputed in bulk with PE matmuls + ACT and
  staged in DRAM in (row, b*T+t) layout.  The recurrence runs as DVE
  tensor_tensor_scan instructions along the time axis (fp32 internal state),
  with partition layout p = b*4 + mq where m = i*4 + mq (i in 0..4 looped).
  The read reduction sum_m w*mem_{t-1} is an elementwise w*traj product (DVE)
  followed by a PE matmul with a constant 0/1 matrix contracting the mq
  partition groups, PSUM-accumulated over the 5 i iterations.
  Final FC: PE matmul over [reads | q | 1] with tanh on ACT.

Engine split in the main loop: Pool builds w*e / w*a products (broadcast
stride-0 views, zero-wait instructions with absorber copies), ACT builds
A = 1-u and evacuations, DVE runs scans + w*traj, PE reduces reads.
"""
import sys

for _p in ("/opt/trn_rl_repo", "/root/.axon_site/_ro/trn_rl_repo"):
    if _p not in sys.path:
        sys.path.append(_p)

import numpy as np
import ml_dtypes

import concourse.bass as bass
import concourse.bacc as bacc
import concourse.tile as tile
from concourse import mybir
from concourse.bass_utils import run_bass_kernel_spmd

B, T, KD, VD, M, FC = 256, 1024, 50, 200, 20, 50
NCORES = 8
BL = B // NCORES          # 32 batch rows per core
MQ, MI = 4, 5             # m = i*MQ + mq
P = BL * MQ               # 128 partitions, p = b*MQ + mq
NBT = BL * T              # 32768 (b, t) columns, col = b*T + t
CH = 512                  # phase-1/3 column chunk
DC = 4                    # phase-2 d-chunk

bf16 = mybir.dt.bfloat16
f32 = mybir.dt.float32
AL = mybir.AluOpType
AF = mybir.ActivationFunctionType


def _ap(tensor, offset, dims):
    return bass.AP(tensor=tensor, offset=offset, ap=list(dims))


def build(do_compile=True):
    nc = bacc.Bacc()

    # ---- external IO (host-prepped layouts) ----
    qT_aug = nc.dram_tensor("qT_aug", [KD + 1, NBT], bf16, kind="ExternalInput")
    qaT = nc.dram_tensor("qaT", [VD, NBT], bf16, kind="ExternalInput")
    kmT = nc.dram_tensor("kmT", [KD, M], bf16, kind="ExternalInput")
    WeT = nc.dram_tensor("WeT", [VD, VD], bf16, kind="ExternalInput")
    WaT = nc.dram_tensor("WaT", [VD, VD], bf16, kind="ExternalInput")
    be = nc.dram_tensor("be", [VD, 1], f32, kind="ExternalInput")
    ba = nc.dram_tensor("ba", [VD, 1], f32, kind="ExternalInput")
    Wr_aug = nc.dram_tensor("Wr_aug", [VD + KD + 1, FC], f32, kind="ExternalInput")
    qTf = nc.dram_tensor("qTf", [KD + 1, NBT], f32, kind="ExternalInput")
    mem0_h = nc.dram_tensor("mem0_h", [P, MI * VD], f32, kind="ExternalInput")
    red_h = nc.dram_tensor("red_h", [P, BL], bf16, kind="ExternalInput")
    ones_h = nc.dram_tensor("ones_h", [M, M], bf16, kind="ExternalInput")
    out = nc.dram_tensor("out", [NBT, FC], f32, kind="ExternalOutput")

    with tile.TileContext(nc) as tc:
        with tc.tile_pool(name="dram", bufs=1, space="DRAM") as dram:
            e_hbm = dram.tile([VD, NBT], bf16)
            a_hbm = dram.tile([VD, NBT], bf16)
            w_hbm = dram.tile([M, NBT], bf16)
            reads_hbm = dram.tile([VD, NBT], f32)

            # ================= phase 1: gates =================
            with tc.tile_pool(name="p1c", bufs=1) as p1c, \
                 tc.tile_pool(name="p1", bufs=3) as p1, \
                 tc.tile_pool(name="p1ps", bufs=2, space="PSUM") as p1ps:
                kmT_sb = p1c.tile([KD, M], bf16)
                nc.sync.dma_start(out=kmT_sb, in_=kmT[:, :])
                ones_sb = p1c.tile([M, M], bf16)
                nc.sync.dma_start(out=ones_sb, in_=ones_h[:, :])
                WeT0 = p1c.tile([128, VD], bf16)
                WeT1 = p1c.tile([VD - 128, VD], bf16)
                WaT0 = p1c.tile([128, VD], bf16)
                WaT1 = p1c.tile([VD - 128, VD], bf16)
                nc.sync.dma_start(out=WeT0, in_=WeT[0:128, :])
                nc.sync.dma_start(out=WeT1, in_=WeT[128:VD, :])
                nc.sync.dma_start(out=WaT0, in_=WaT[0:128, :])
                nc.sync.dma_start(out=WaT1, in_=WaT[128:VD, :])
                be0 = p1c.tile([128, 1], f32)
                be1 = p1c.tile([VD - 128, 1], f32)
                ba0 = p1c.tile([128, 1], f32)
                ba1 = p1c.tile([VD - 128, 1], f32)
                nc.sync.dma_start(out=be0, in_=be[0:128, :])
                nc.sync.dma_start(out=be1, in_=be[128:VD, :])
                nc.sync.dma_start(out=ba0, in_=ba[0:128, :])
                nc.sync.dma_start(out=ba1, in_=ba[128:VD, :])

                for c in range(NBT // CH):
                    cs = c * CH
                    # ---- w = softmax(q @ km^T) over m, m on partitions ----
                    qT_t = p1.tile([KD, CH], bf16)
                    nc.sync.dma_start(out=qT_t, in_=qT_aug[0:KD, cs:cs + CH])
                    lg_ps = p1ps.tile([M, CH], f32)
                    nc.tensor.matmul(lg_ps, kmT_sb, qT_t, start=True, stop=True)
                    exp_t = p1.tile([M, CH], bf16)
                    nc.scalar.activation(exp_t, lg_ps, AF.Exp)
                    sum_ps = p1ps.tile([M, CH], f32)
                    nc.tensor.matmul(sum_ps, ones_sb, exp_t, start=True, stop=True)
                    rec_t = p1.tile([M, CH], f32)
                    nc.vector.reciprocal(rec_t, sum_ps)
                    w_t = p1.tile([M, CH], bf16)
                    nc.vector.tensor_mul(w_t, exp_t, rec_t)
                    nc.sync.dma_start(out=w_hbm[:, cs:cs + CH], in_=w_t)

                    # ---- e = sigmoid(qa@We^T+be), a = tanh(qa@Wa^T+ba) ----
                    qa0_t = p1.tile([128, CH], bf16)
                    qa1_t = p1.tile([VD - 128, CH], bf16)
                    nc.sync.dma_start(out=qa0_t, in_=qaT[0:128, cs:cs + CH])
                    nc.sync.dma_start(out=qa1_t, in_=qaT[128:VD, cs:cs + CH])
                    for W0, W1, bb0, bb1, dst, fn in (
                        (WeT0, WeT1, be0, be1, e_hbm, AF.Sigmoid),
                        (WaT0, WaT1, ba0, ba1, a_hbm, AF.Tanh),
                    ):
                        for o0, osz in ((0, 128), (128, VD - 128)):
                            g_ps = p1ps.tile([128, CH], f32, tag="gps")
                            nc.tensor.matmul(g_ps[:osz], W0[:, o0:o0 + osz],
                                             qa0_t, start=True, stop=False)
                            nc.tensor.matmul(g_ps[:osz], W1[:, o0:o0 + osz],
                                             qa1_t, start=False, stop=True)
                            g_t = p1.tile([128, CH], bf16, tag="gsb")
                            bb = bb0 if o0 == 0 else bb1
                            nc.scalar.activation(g_t[:osz], g_ps[:osz], fn,
                                                 bias=bb[:, 0:1])
                            nc.sync.dma_start(out=dst[o0:o0 + osz, cs:cs + CH],
                                              in_=g_t[:osz])

            # ================= phase 2: recurrence =================
            with tc.tile_pool(name="p2c", bufs=1) as p2c, \
                 tc.tile_pool(name="ea", bufs=2) as eap, \
                 tc.tile_pool(name="ue", bufs=2) as uep, \
                 tc.tile_pool(name="aa", bufs=3) as aap, \
                 tc.tile_pool(name="tj", bufs=2) as tjp, \
                 tc.tile_pool(name="wp", bufs=2) as wpp, \
                 tc.tile_pool(name="rs", bufs=3) as rsp, \
                 tc.tile_pool(name="ab", bufs=2) as abp, \
                 tc.tile_pool(name="p2ps", bufs=2, space="PSUM") as p2ps:
                w_buf = p2c.tile([P, MI, T], bf16)
                # w_hbm[m, b*T+t] -> partition p=mq*BL+b, free (i, t); m=i*MQ+mq
                for mq in range(MQ):
                    nc.sync.dma_start(
                        out=w_buf[mq * BL:(mq + 1) * BL, :, :],
                        in_=_ap(w_hbm.tensor, w_hbm.offset + mq * NBT,
                                [[T, BL], [MQ * NBT, MI], [1, T]]))
                mem0_sb = p2c.tile([P, MI * VD], f32)
                nc.sync.dma_start(out=mem0_sb, in_=mem0_h[:, :])
                red_sb = p2c.tile([P, BL], bf16)
                nc.sync.dma_start(out=red_sb, in_=red_h[:, :])

                # tiles whose producers Pool must observe before its zero-wait
                # compute ops; absorb at d-2 depth to keep the pipeline deep
                hist_dve = [w_buf[:, 0, 0:1], w_buf[:, 0, 0:1]]
                hist_act = [mem0_sb[:, 0:1], mem0_sb[:, 0:1]]
                for dchunk in range(VD // DC):
                    d0 = dchunk * DC
                    ea_t = eap.tile([P, DC, 2, T], bf16)
                    # e_hbm[d, b*T+t] -> [p=mq*BL+b (mq-replicated), (dj, slot, t)]
                    for slot, src in ((0, e_hbm), (1, a_hbm)):
                        for mq in range(MQ):
                            nc.sync.dma_start(
                                out=ea_t[mq * BL:(mq + 1) * BL, :, slot, :],
                                in_=_ap(src.tensor, src.offset + d0 * NBT,
                                        [[T, BL], [NBT, DC], [1, T]]))
                    # Pool absorbers for the two chunk DMAs
                    ab_t = abp.tile([P, 2], bf16, tag="ab")
                    nc.gpsimd.tensor_copy(ab_t[:, 0:1], ea_t[:, 0, 0, 0:1])
                    nc.gpsimd.tensor_copy(ab_t[:, 1:2], ea_t[:, 0, 1, 0:1])

                    for dj in range(DC):
                        d = d0 + dj
                        # Pool absorbers: observe d-2 DVE + ACT ticks so Pool
                        # compute ops need zero attached waits without
                        # serializing against the previous d's tail
                        ab2_t = abp.tile([P, 2], bf16, tag="ab2")
                        hd, ha = hist_dve[0], hist_act[0]
                        nc.gpsimd.tensor_copy(ab2_t[0:hd.shape[0], 0:1], hd)
                        nc.gpsimd.tensor_copy(ab2_t[0:ha.shape[0], 1:2], ha)
                        # ueua[p, i, 0, t] = w*e ; [p, i, 1, t] = w*a
                        # split: Pool builds i in [0, NPI), DVE the rest;
                        # emit the e-halves first so ACT's 1-u starts early
                        NPI = 2 + (d % 2)
                        NDI = MI - NPI
                        ueua = uep.tile([P, MI, 2, T], bf16)
                        e_v = ea_t[:, dj, 0, :].unsqueeze(1)
                        a_v = ea_t[:, dj, 1, :].unsqueeze(1)
                        nc.gpsimd.tensor_mul(
                            ueua[:, 0:NPI, 0, :], w_buf[:, 0:NPI, :],
                            e_v.broadcast_to([P, NPI, T]))
                        nc.vector.tensor_mul(
                            ueua[:, NPI:MI, 0, :], w_buf[:, NPI:MI, :],
                            e_v.broadcast_to([P, NDI, T]))
                        nc.gpsimd.tensor_mul(
                            ueua[:, 0:NPI, 1, :], w_buf[:, 0:NPI, :],
                            a_v.broadcast_to([P, NPI, T]))
                        nc.vector.tensor_mul(
                            ueua[:, NPI:MI, 1, :], w_buf[:, NPI:MI, :],
                            a_v.broadcast_to([P, NDI, T]))

                        # A = 1 - u   (ACT, fp32 to limit compounding error)
                        A_all = aap.tile([P, MI, T], f32)
                        nc.scalar.activation(A_all, ueua[:, :, 0, :], AF.Copy,
                                             bias=1.0, scale=-1.0)

                        read_ps = p2ps.tile([BL, T], f32)
                        traj_all = tjp.tile([P, MI, T + 1], bf16, tag="tj")
                        m0_v = _ap(mem0_sb.tensor, mem0_sb.offset + d,
                                   [list(mem0_sb.ap[0]), [VD, MI]])
                        nc.scalar.copy(traj_all[:, :, 0], m0_v)
                        for i in range(MI):
                            col = i * VD + d
                            nc.vector.tensor_tensor_scan(
                                out=traj_all[:, i, 1:T + 1],
                                data0=A_all[:, i, :],
                                data1=ueua[:, i, 1, :],
                                initial=mem0_sb[:, col:col + 1],
                                op0=AL.mult, op1=AL.add)
                        wp_all = wpp.tile([P, MI, T], bf16, tag="wp")
                        nc.vector.tensor_mul(wp_all, w_buf[:, :, :],
                                             traj_all[:, :, 0:T])
                        for i in range(MI):
                            for h in range(T // 512):
                                nc.tensor.matmul(
                                    read_ps[:, h * 512:(h + 1) * 512],
                                    red_sb,
                                    wp_all[:, i, h * 512:(h + 1) * 512],
                                    start=(i == 0), stop=(i == MI - 1))
                        wp_last = wp_all[:, 0, :]
                        read_sb = rsp.tile([BL, T], f32)
                        nc.scalar.copy(read_sb, read_ps)
                        nc.sync.dma_start(
                            out=_ap(reads_hbm.tensor,
                                    reads_hbm.offset + d * NBT,
                                    [[T, BL], [1, T]]),
                            in_=read_sb)
                        hist_dve = [hist_dve[1], wp_last[:, 0:1]]
                        hist_act = [hist_act[1], read_sb[:, 0:1]]

            # ================= phase 3: final FC =================
            with tc.tile_pool(name="p3c", bufs=1) as p3c, \
                 tc.tile_pool(name="p3", bufs=3) as p3, \
                 tc.tile_pool(name="p3ps", bufs=4, space="PSUM") as p3ps:
                Wr0 = p3c.tile([128, FC], f32)
                Wr1 = p3c.tile([VD - 128, FC], f32)
                Wr2 = p3c.tile([KD + 1, FC], f32)
                nc.sync.dma_start(out=Wr0, in_=Wr_aug[0:128, :])
                nc.sync.dma_start(out=Wr1, in_=Wr_aug[128:VD, :])
                nc.sync.dma_start(out=Wr2, in_=Wr_aug[VD:VD + KD + 1, :])
                for c in range(NBT // 128):
                    cs = c * 128
                    r0 = p3.tile([128, 128], f32, tag="r0")
                    r1 = p3.tile([VD - 128, 128], f32, tag="r1")
                    qf = p3.tile([KD + 1, 128], f32, tag="qf")
                    nc.sync.dma_start(out=r0, in_=reads_hbm[0:128, cs:cs + 128])
                    nc.sync.dma_start(out=r1, in_=reads_hbm[128:VD, cs:cs + 128])
                    nc.sync.dma_start(out=qf, in_=qTf[:, cs:cs + 128])
                    o_ps = p3ps.tile([128, FC], f32)
                    nc.tensor.matmul(o_ps, r0, Wr0, start=True, stop=False)
                    nc.tensor.matmul(o_ps, r1, Wr1, start=False, stop=False)
                    nc.tensor.matmul(o_ps, qf, Wr2, start=False, stop=True)
                    o_t = p3.tile([128, FC], f32, tag="ot")
                    nc.scalar.activation(o_t, o_ps, AF.Tanh)
                    nc.sync.dma_start(out=out[cs:cs + 128, :], in_=o_t)

    if do_compile:
        nc.compile()
    return nc


_NC_CACHE = None


def _get_nc():
    global _NC_CACHE
    if _NC_CACHE is None:
        _NC_CACHE = build()
    return _NC_CACHE


def _prep_core_inputs(q_c, qa_c, key_memory, init_value_memory,
                      W_erase, b_erase, W_add, b_add, W_read, b_read):
    """q_c: (BL, T, KD) f32; qa_c: (BL, T, VD) f32."""
    b16 = ml_dtypes.bfloat16
    qT = np.ascontiguousarray(q_c.reshape(NBT, KD).T)          # [KD, NBT]
    qT_aug = np.concatenate([qT, np.ones((1, NBT), np.float32)], 0)
    qaT = np.ascontiguousarray(qa_c.reshape(NBT, VD).T)        # [VD, NBT]
    # mem0_sb[p=mq*BL+b, i*VD+d] = ivm[i*MQ+mq, d]
    ivm = init_value_memory.reshape(MI, MQ, VD).transpose(1, 0, 2)  # [mq, i, d]
    mem0 = np.tile(ivm.reshape(MQ, 1, MI * VD), (1, BL, 1)).reshape(P, MI * VD)
    red = np.zeros((P, BL), np.float32)
    red[np.arange(P), np.arange(P) % BL] = 1.0
    Wr_aug = np.concatenate([W_read.T, b_read[None, :]], 0)    # [251, FC]
    return {
        "qT_aug": qT_aug.astype(b16),
        "qaT": qaT.astype(b16),
        "kmT": np.ascontiguousarray(key_memory.T).astype(b16),
        "WeT": np.ascontiguousarray(W_erase.T).astype(b16),
        "WaT": np.ascontiguousarray(W_add.T).astype(b16),
        "be": b_erase.reshape(VD, 1).astype(np.float32),
        "ba": b_add.reshape(VD, 1).astype(np.float32),
        "Wr_aug": Wr_aug.astype(np.float32),
        "qTf": qT_aug.astype(np.float32),
        "mem0_h": mem0.astype(np.float32),
        "red_h": red.astype(b16),
        "ones_h": np.ones((M, M), np.float32).astype(b16),
    }


def kernel(q_embed, qa_embed, key_memory, init_value_memory,
           W_erase, b_erase, W_add, b_add, W_read, b_read):
    args = [np.asarray(x) for x in
            (key_memory, init_value_memory, W_erase, b_erase,
             W_add, b_add, W_read, b_read)]
    q_embed = np.asarray(q_embed, np.float32)
    qa_embed = np.asarray(qa_embed, np.float32)
    nc = _get_nc()
    in_maps = []
    for c in range(NCORES):
        sl = slice(c * BL, (c + 1) * BL)
        in_maps.append(_prep_core_inputs(q_embed[sl], qa_embed[sl], *args))
    res = run_bass_kernel_spmd(nc, in_maps, list(range(NCORES)))
    outs = [res.results[c]["out"].reshape(BL, T, FC) for c in range(NCORES)]
    return np.concatenate(outs, axis=0).astype(np.float32)


if __name__ == "__main__":
    # smoke: random tiny check against a numpy reference
    rng = np.random.default_rng(0)
    q = rng.standard_normal((B, T, KD)).astype(np.float32)
    qa = rng.standard_normal((B, T, VD)).astype(np.float32)
    km = (rng.standard_normal((M, KD)) * 0.2).astype(np.float32)
    ivm = (rng.standard_normal((M, VD)) * 0.1).astype(np.float32)
    We = (rng.standard_normal((VD, VD)) * 0.1).astype(np.float32)
    Wa = (rng.standard_normal((VD, VD)) * 0.1).astype(np.float32)
    Wr = (rng.standard_normal((FC, VD + KD)) * 0.09).astype(np.float32)
    o = kernel(q, qa, km, ivm, We, np.zeros(VD, np.float32), Wa,
               np.zeros(VD, np.float32), Wr, np.zeros(FC, np.float32))
    print("out", o.shape, o.dtype, float(np.abs(o).mean()))
